# revision 3
# baseline (speedup 1.0000x reference)
"""GCN-style channel propagation (gather + normalized segment-sum) on 8 TRN2
NeuronCores.

    out = D^{-1/2} A D^{-1/2} x      (A from edge_index COO, deg = in-degree)

Sharding: edges are sharded by destination node across the 8 cores (each core
owns a contiguous range of destination nodes, so partial outputs are disjoint
and no all-reduce is needed).  Within a core, edges are bucketed by
(destination window of 128 nodes, source half) and streamed as 128-slot tiles:

  dma_gather      (SWDGE)  pulls x[col] rows (padded to 512B) from HBM to SBUF
  tensor_scalar   (DVE/ACT) builds S[p, m] = (iota[m] == seg[p]) * norm[p]
  matmul          (PE)     accumulates S.T @ gathered into a PSUM window
  tensor_copy     (DVE)    drains each 128-node PSUM window
  dma_start       (SP)     writes the window to the per-core output

The source side is split in two halves so gather indices fit int16 (the
dma_gather index dtype); both halves accumulate into the same PSUM window.
Host-side work is limited to index bookkeeping (bucketing/padding the edge
list, degree counts and the per-edge norm coefficients, i.e. the static
graph preprocessing a GNN framework caches per graph).
"""

import math

import numpy as np

import concourse.bacc as bacc
import concourse.mybir as mybir
import concourse.tile as tile
from concourse import bass_utils

N_CORES = 8
P = 128
F_OUT = 96  # feature dim of the problem
F_PAD = 128  # gather element (512B) — <512B DMA descriptors pay a 2x penalty
WIN_PER_SUPER = 4  # PSUM windows per gather super-chunk (8 banks / 2 buffers)

_TRACE = {"on": False, "kwargs": {}}
_LAST = {}


def _ceil_div(a, b):
    return (a + b - 1) // b


def _preprocess(x, edge_index, num_nodes):
    """Host-side index bookkeeping: shard/bucket/pad the edge list and build
    the per-core gather/seg/norm streams plus the static tile schedule."""
    n = int(num_nodes)
    row = np.asarray(edge_index[0], dtype=np.int64)
    col = np.asarray(edge_index[1], dtype=np.int64)

    npc = _ceil_div(n, N_CORES)  # nodes per core
    n_win = _ceil_div(npc, P)  # destination windows per core
    half = _ceil_div(n, 2)  # source half size (must fit int16)
    assert half <= 32767, f"source half {half} does not fit int16"

    # degree-based symmetric normalization (deg = in-degree as destination)
    deg = np.bincount(row, minlength=n).astype(np.float32)
    dis = np.where(deg > 0, 1.0 / np.sqrt(np.maximum(deg, 1.0)), 0.0).astype(
        np.float32
    )
    norm_e = (dis[row] * dis[col]).astype(np.float32)

    core = row // npc
    d_loc = row - core * npc
    win = d_loc // P
    seg = (d_loc % P).astype(np.float32)
    h = (col >= half).astype(np.int64)
    idx16 = (col - h * half).astype(np.int16)

    # bucket key: (core, window, half) — order within a bucket is irrelevant
    key = (core * n_win + win) * 2 + h
    order = np.argsort(key, kind="stable")
    key_s = key[order]
    counts = np.bincount(key_s, minlength=N_CORES * n_win * 2).reshape(
        N_CORES, n_win, 2
    )
    # uniform SPMD program: every core pads bucket (w, h) to the max tile count
    tiles_wh = _ceil_div(counts.max(axis=0), P)  # [n_win, 2]
    slots_wh = tiles_wh * P

    # stream layout: supers of WIN_PER_SUPER windows; within a super all lo
    # slots of its windows, then all hi slots (one gather instruction each)
    n_super = _ceil_div(n_win, WIN_PER_SUPER)
    bucket_off = np.zeros((n_win, 2), dtype=np.int64)  # slot offset of bucket
    cursor = 0
    for s in range(n_super):
        ws = range(s * WIN_PER_SUPER, min((s + 1) * WIN_PER_SUPER, n_win))
        for hh in (0, 1):
            for w in ws:
                bucket_off[w, hh] = cursor
                cursor += slots_wh[w, hh]
    s_total = int(cursor)
    t_total = s_total // P

    # per-core streams
    idx_stream = np.zeros((N_CORES, s_total), dtype=np.int16)
    seg_stream = np.full((N_CORES, s_total), -1.0, dtype=np.float32)
    norm_stream = np.zeros((N_CORES, s_total), dtype=np.float32)

    starts = np.zeros(N_CORES * n_win * 2 + 1, dtype=np.int64)
    np.cumsum(counts.reshape(-1), out=starts[1:])
    for c in range(N_CORES):
        for w in range(n_win):
            for hh in (0, 1):
                k = (c * n_win + w) * 2 + hh
                sel = order[starts[k] : starts[k + 1]]
                o = bucket_off[w, hh]
                m = sel.size
                idx_stream[c, o : o + m] = idx16[sel]
                seg_stream[c, o : o + m] = seg[sel]
                norm_stream[c, o : o + m] = norm_e[sel]

    # device layouts
    # gather idx: slot i -> partition i%16, col i//16 (replicated to 128 rows)
    idx_packed = np.tile(
        idx_stream.reshape(N_CORES, s_total // 16, 16).transpose(0, 2, 1),
        (1, 8, 1),
    ).copy()  # [cores, 128, s_total/16]
    # seg/norm: slot i -> partition i%128, col i//128
    seg_cols = seg_stream.reshape(N_CORES, t_total, P).transpose(0, 2, 1).copy()
    norm_cols = norm_stream.reshape(N_CORES, t_total, P).transpose(0, 2, 1).copy()

    # x padded to 512B rows, split in two halves (dma_gather ignores AP
    # base offsets on HW, so each half must be its own DRAM tensor)
    x = np.asarray(x, dtype=np.float32)
    x_pad = np.zeros((n, F_PAD), dtype=np.float32)
    x_pad[:, :F_OUT] = x
    x_lo = np.ascontiguousarray(x_pad[:half])
    x_hi = np.ascontiguousarray(x_pad[half:])

    sched = {
        "n": n,
        "npc": npc,
        "n_win": n_win,
        "half": half,
        "n_rows_lo": half,
        "n_rows_hi": n - half,
        "tiles_wh": tiles_wh,
        "bucket_off": bucket_off,
        "s_total": s_total,
        "t_total": t_total,
    }
    return sched, x_lo, x_hi, idx_packed, seg_cols, norm_cols


def _build_program(sched):
    n = sched["n"]
    n_win = sched["n_win"]
    half = sched["half"]
    tiles_wh = sched["tiles_wh"]
    bucket_off = sched["bucket_off"]
    s_total = sched["s_total"]
    t_total = sched["t_total"]
    out_rows = n_win * P

    nc = bacc.Bacc("TRN2", target_bir_lowering=False)
    x_lo_d = nc.dram_tensor(
        "x_lo", [half, F_PAD], mybir.dt.float32, kind="ExternalInput"
    )
    x_hi_d = nc.dram_tensor(
        "x_hi", [n - half, F_PAD], mybir.dt.float32, kind="ExternalInput"
    )
    idx_d = nc.dram_tensor(
        "idx", [P, s_total // 16], mybir.dt.int16, kind="ExternalInput"
    )
    seg_d = nc.dram_tensor("seg", [P, t_total], mybir.dt.float32, kind="ExternalInput")
    norm_d = nc.dram_tensor(
        "norm", [P, t_total], mybir.dt.float32, kind="ExternalInput"
    )
    out_d = nc.dram_tensor(
        "out", [out_rows, F_OUT], mybir.dt.float32, kind="ExternalOutput"
    )

    n_super = _ceil_div(n_win, WIN_PER_SUPER)

    with tile.TileContext(nc) as tc:
        with tc.tile_pool(name="meta", bufs=1) as meta, tc.tile_pool(
            name="gat", bufs=2
        ) as gat, tc.tile_pool(name="s", bufs=4) as sp, tc.tile_pool(
            name="o", bufs=3
        ) as op, tc.tile_pool(name="ps", bufs=8, space="PSUM") as pp:
            idx_t = meta.tile([P, s_total // 16], mybir.dt.int16)
            nc.sync.dma_start(idx_t[:], idx_d[:])
            seg_t = meta.tile([P, t_total], mybir.dt.float32)
            nc.sync.dma_start(seg_t[:], seg_d[:])
            norm_t = meta.tile([P, t_total], mybir.dt.float32)
            nc.sync.dma_start(norm_t[:], norm_d[:])

            iota_i = meta.tile([P, P], mybir.dt.int32)
            nc.gpsimd.iota(iota_i[:], pattern=[[1, P]], base=0, channel_multiplier=0)
            iota_f = meta.tile([P, P], mybir.dt.float32)
            nc.vector.tensor_copy(iota_f[:], iota_i[:])

            for s in range(n_super):
                ws = list(range(s * WIN_PER_SUPER, min((s + 1) * WIN_PER_SUPER, n_win)))
                # one gather per (super, half)
                gbuf = {}
                for hh, src in ((0, x_lo_d[:]), (1, x_hi_d[:])):
                    n_slots = int(sum(tiles_wh[w, hh] for w in ws)) * P
                    if n_slots == 0:
                        continue
                    o = int(bucket_off[ws[0], hh])
                    g = gat.tile(
                        [P, n_slots // P, F_PAD],
                        mybir.dt.float32,
                        tag=f"g{hh}",
                    )
                    nc.gpsimd.dma_gather(
                        g[:],
                        src,
                        idx_t[:, o // 16 : (o + n_slots) // 16],
                        n_slots,
                        n_slots,
                        F_PAD,
                        single_packet=False,
                    )
                    gbuf[hh] = (g, o // P)  # tile-column base of this chunk

                for w in ws:
                    psum = pp.tile([P, F_OUT], mybir.dt.float32, space="PSUM")
                    # (gather buffer, local col, global tile) per tile of w
                    tl = []
                    for hh in (0, 1):
                        tw = int(tiles_wh[w, hh])
                        if tw == 0:
                            continue
                        g, base = gbuf[hh]
                        t0 = int(bucket_off[w, hh]) // P
                        for j in range(tw):
                            tl.append((g, t0 - base + j, t0 + j))
                    outsb = op.tile([P, F_OUT], mybir.dt.float32)
                    if not tl:
                        nc.vector.memset(outsb[:], 0.0)
                    else:
                        for k, (g, jloc, tg) in enumerate(tl):
                            s_t = sp.tile([P, P], mybir.dt.float32, tag="s")
                            nc.any.tensor_scalar(
                                out=s_t[:],
                                in0=iota_f[:],
                                scalar1=seg_t[:, tg : tg + 1],
                                scalar2=norm_t[:, tg : tg + 1],
                                op0=mybir.AluOpType.is_equal,
                                op1=mybir.AluOpType.mult,
                            )
                            nc.tensor.matmul(
                                out=psum[:],
                                lhsT=s_t[:],
                                rhs=g[:, jloc, 0:F_OUT],
                                start=(k == 0),
                                stop=(k == len(tl) - 1),
                            )
                        nc.vector.tensor_copy(outsb[:], psum[:])
                    nc.sync.dma_start(out_d[w * P : (w + 1) * P, :], outsb[:])

    nc.compile()
    return nc


def kernel(x, edge_index, num_nodes):
    sched, x_lo, x_hi, idx_packed, seg_cols, norm_cols = _preprocess(
        x, edge_index, num_nodes
    )
    nc = _build_program(sched)

    in_maps = [
        {
            "x_lo": x_lo,
            "x_hi": x_hi,
            "idx": idx_packed[c],
            "seg": seg_cols[c],
            "norm": norm_cols[c],
        }
        for c in range(N_CORES)
    ]
    res = bass_utils.run_bass_kernel_spmd(
        nc,
        in_maps,
        core_ids=list(range(N_CORES)),
        trace=_TRACE["on"],
        **_TRACE["kwargs"],
    )
    _LAST["results"] = res

    n = sched["n"]
    npc = sched["npc"]
    parts = []
    for c in range(N_CORES):
        lo = c * npc
        hi = min(n, lo + npc)
        if hi > lo:
            parts.append(res.results[c]["out"][: hi - lo])
    out = np.concatenate(parts, axis=0)
    return np.ascontiguousarray(out, dtype=np.float32)


# revision 4
# speedup vs baseline: 1.1705x; 1.1705x over previous
"""GCN-style channel propagation (gather + normalized segment-sum) on 8 TRN2
NeuronCores.

    out = D^{-1/2} A D^{-1/2} x      (A from edge_index COO, deg = in-degree)

Sharding: edges are sharded by destination node across the 8 cores (each core
owns a contiguous range of destination nodes, so partial outputs are disjoint
and no all-reduce is needed).  Within a core, edges are bucketed by
(destination window of 128 nodes, source half) and streamed as 128-slot tiles:

  dma_gather      (SWDGE)  pulls x[col] rows (padded to 512B) from HBM to SBUF
  tensor_scalar   (DVE/ACT) builds S[p, m] = (iota[m] == seg[p]) * norm[p]
  matmul          (PE)     accumulates S.T @ gathered into a PSUM window
  tensor_copy     (DVE)    drains each 128-node PSUM window
  dma_start       (SP)     writes the window to the per-core output

The source side is split in two halves so gather indices fit int16 (the
dma_gather index dtype); both halves accumulate into the same PSUM window.
Host-side work is limited to index bookkeeping (bucketing/padding the edge
list, degree counts and the per-edge norm coefficients, i.e. the static
graph preprocessing a GNN framework caches per graph).
"""

import math

import numpy as np

import concourse.bacc as bacc
import concourse.mybir as mybir
import concourse.tile as tile
from concourse import bass_utils

N_CORES = 8
P = 128
F_OUT = 96  # feature dim of the problem
F_PAD = 128  # gather element (512B) — <512B DMA descriptors pay a 2x penalty
WIN_PER_SUPER = 4  # PSUM windows per gather super-chunk (8 banks / 2 buffers)

_TRACE = {"on": False, "kwargs": {}}
_LAST = {}


def _ceil_div(a, b):
    return (a + b - 1) // b


def _preprocess(x, edge_index, num_nodes):
    """Host-side index bookkeeping: shard/bucket/pad the edge list and build
    the per-core gather/seg/norm streams plus the static tile schedule."""
    n = int(num_nodes)
    row = np.asarray(edge_index[0], dtype=np.int64)
    col = np.asarray(edge_index[1], dtype=np.int64)

    npc = _ceil_div(n, N_CORES)  # nodes per core
    n_win = _ceil_div(npc, P)  # destination windows per core
    half = _ceil_div(n, 2)  # source half size (must fit int16)
    assert half <= 32767, f"source half {half} does not fit int16"

    # degree-based symmetric normalization (deg = in-degree as destination)
    deg = np.bincount(row, minlength=n).astype(np.float32)
    dis = np.where(deg > 0, 1.0 / np.sqrt(np.maximum(deg, 1.0)), 0.0).astype(
        np.float32
    )
    norm_e = (dis[row] * dis[col]).astype(np.float32)

    # degree-snake node placement: sort nodes by degree and deal them
    # round-robin across cores/windows so every (core, window) bucket has a
    # near-equal edge count (minimizes SPMD padding).  The host unshard
    # applies the inverse permutation.
    order_nodes = np.argsort(-deg, kind="stable").astype(np.int64)
    rank = np.empty(n, dtype=np.int64)
    rank[order_nodes] = np.arange(n, dtype=np.int64)
    node_core = rank % N_CORES
    node_pos = rank // N_CORES  # output row within its core

    core = node_core[row]
    d_loc = node_pos[row]
    win = d_loc // P
    seg = (d_loc % P).astype(np.float32)
    h = (col >= half).astype(np.int64)
    idx16 = (col - h * half).astype(np.int16)

    # bucket key: (core, window, half) — order within a bucket is irrelevant
    key = (core * n_win + win) * 2 + h
    order = np.argsort(key, kind="stable")
    key_s = key[order]
    counts = np.bincount(key_s, minlength=N_CORES * n_win * 2).reshape(
        N_CORES, n_win, 2
    )
    # uniform SPMD program: every core pads bucket (w, h) to the max tile count
    tiles_wh = _ceil_div(counts.max(axis=0), P)  # [n_win, 2]
    slots_wh = tiles_wh * P

    # stream layout: supers of WIN_PER_SUPER windows; within a super all lo
    # slots of its windows, then all hi slots (one gather instruction each)
    n_super = _ceil_div(n_win, WIN_PER_SUPER)
    bucket_off = np.zeros((n_win, 2), dtype=np.int64)  # slot offset of bucket
    cursor = 0
    for s in range(n_super):
        ws = range(s * WIN_PER_SUPER, min((s + 1) * WIN_PER_SUPER, n_win))
        for hh in (0, 1):
            for w in ws:
                bucket_off[w, hh] = cursor
                cursor += slots_wh[w, hh]
    s_total = int(cursor)
    t_total = s_total // P

    # per-core streams
    idx_stream = np.zeros((N_CORES, s_total), dtype=np.int16)
    seg_stream = np.full((N_CORES, s_total), -1.0, dtype=np.float32)
    norm_stream = np.zeros((N_CORES, s_total), dtype=np.float32)

    starts = np.zeros(N_CORES * n_win * 2 + 1, dtype=np.int64)
    np.cumsum(counts.reshape(-1), out=starts[1:])
    for c in range(N_CORES):
        for w in range(n_win):
            for hh in (0, 1):
                k = (c * n_win + w) * 2 + hh
                sel = order[starts[k] : starts[k + 1]]
                o = bucket_off[w, hh]
                m = sel.size
                idx_stream[c, o : o + m] = idx16[sel]
                seg_stream[c, o : o + m] = seg[sel]
                norm_stream[c, o : o + m] = norm_e[sel]

    # device layouts
    # gather idx: slot i -> partition i%16, col i//16 (replicated to 128 rows)
    idx_packed = np.tile(
        idx_stream.reshape(N_CORES, s_total // 16, 16).transpose(0, 2, 1),
        (1, 8, 1),
    ).copy()  # [cores, 128, s_total/16]
    # seg/norm: slot i -> partition i%128, col i//128
    seg_cols = seg_stream.reshape(N_CORES, t_total, P).transpose(0, 2, 1).copy()
    norm_cols = norm_stream.reshape(N_CORES, t_total, P).transpose(0, 2, 1).copy()

    # x padded to 512B rows, split in two halves (dma_gather ignores AP
    # base offsets on HW, so each half must be its own DRAM tensor)
    x = np.asarray(x, dtype=np.float32)
    x_pad = np.zeros((n, F_PAD), dtype=np.float32)
    x_pad[:, :F_OUT] = x
    x_lo = np.ascontiguousarray(x_pad[:half])
    x_hi = np.ascontiguousarray(x_pad[half:])

    sched = {
        "n": n,
        "npc": npc,
        "n_win": n_win,
        "half": half,
        "n_rows_lo": half,
        "n_rows_hi": n - half,
        "tiles_wh": tiles_wh,
        "bucket_off": bucket_off,
        "s_total": s_total,
        "t_total": t_total,
        "order_nodes": order_nodes,
    }
    return sched, x_lo, x_hi, idx_packed, seg_cols, norm_cols


def _build_program(sched):
    n = sched["n"]
    n_win = sched["n_win"]
    half = sched["half"]
    tiles_wh = sched["tiles_wh"]
    bucket_off = sched["bucket_off"]
    s_total = sched["s_total"]
    t_total = sched["t_total"]
    out_rows = n_win * P

    nc = bacc.Bacc("TRN2", target_bir_lowering=False, num_swdge_queues=4)
    x_lo_d = nc.dram_tensor(
        "x_lo", [half, F_PAD], mybir.dt.float32, kind="ExternalInput"
    )
    x_hi_d = nc.dram_tensor(
        "x_hi", [n - half, F_PAD], mybir.dt.float32, kind="ExternalInput"
    )
    idx_d = nc.dram_tensor(
        "idx", [P, s_total // 16], mybir.dt.int16, kind="ExternalInput"
    )
    seg_d = nc.dram_tensor("seg", [P, t_total], mybir.dt.float32, kind="ExternalInput")
    norm_d = nc.dram_tensor(
        "norm", [P, t_total], mybir.dt.float32, kind="ExternalInput"
    )
    out_d = nc.dram_tensor(
        "out", [out_rows, F_OUT], mybir.dt.float32, kind="ExternalOutput"
    )

    n_super = _ceil_div(n_win, WIN_PER_SUPER)

    with tile.TileContext(nc) as tc:
        with tc.tile_pool(name="meta", bufs=1) as meta, tc.tile_pool(
            name="gat", bufs=2
        ) as gat, tc.tile_pool(name="s", bufs=4) as sp, tc.tile_pool(
            name="o", bufs=3
        ) as op, tc.tile_pool(name="ps", bufs=8, space="PSUM") as pp:
            idx_t = meta.tile([P, s_total // 16], mybir.dt.int16)
            nc.sync.dma_start(idx_t[:], idx_d[:])
            seg_t = meta.tile([P, t_total], mybir.dt.float32)
            nc.sync.dma_start(seg_t[:], seg_d[:])
            norm_t = meta.tile([P, t_total], mybir.dt.float32)
            nc.sync.dma_start(norm_t[:], norm_d[:])

            iota_i = meta.tile([P, P], mybir.dt.int32)
            nc.gpsimd.iota(iota_i[:], pattern=[[1, P]], base=0, channel_multiplier=0)
            iota_f = meta.tile([P, P], mybir.dt.float32)
            nc.vector.tensor_copy(iota_f[:], iota_i[:])

            for s in range(n_super):
                ws = list(range(s * WIN_PER_SUPER, min((s + 1) * WIN_PER_SUPER, n_win)))
                # one gather per (super, half)
                gbuf = {}
                for hh, src in ((0, x_lo_d[:]), (1, x_hi_d[:])):
                    n_slots = int(sum(tiles_wh[w, hh] for w in ws)) * P
                    if n_slots == 0:
                        continue
                    o = int(bucket_off[ws[0], hh])
                    g = gat.tile(
                        [P, n_slots // P, F_PAD],
                        mybir.dt.float32,
                        tag=f"g{hh}",
                    )
                    nc.gpsimd.dma_gather(
                        g[:],
                        src,
                        idx_t[:, o // 16 : (o + n_slots) // 16],
                        n_slots,
                        n_slots,
                        F_PAD,
                        single_packet=False,
                        queue_num=(2 * s + hh) % 4,
                    )
                    gbuf[hh] = (g, o // P)  # tile-column base of this chunk

                for w in ws:
                    psum = pp.tile([P, F_OUT], mybir.dt.float32, space="PSUM")
                    # (gather buffer, local col, global tile) per tile of w
                    tl = []
                    for hh in (0, 1):
                        tw = int(tiles_wh[w, hh])
                        if tw == 0:
                            continue
                        g, base = gbuf[hh]
                        t0 = int(bucket_off[w, hh]) // P
                        for j in range(tw):
                            tl.append((g, t0 - base + j, t0 + j))
                    outsb = op.tile([P, F_OUT], mybir.dt.float32)
                    if not tl:
                        nc.vector.memset(outsb[:], 0.0)
                    else:
                        for k, (g, jloc, tg) in enumerate(tl):
                            s_t = sp.tile([P, P], mybir.dt.float32, tag="s")
                            nc.any.tensor_scalar(
                                out=s_t[:],
                                in0=iota_f[:],
                                scalar1=seg_t[:, tg : tg + 1],
                                scalar2=norm_t[:, tg : tg + 1],
                                op0=mybir.AluOpType.is_equal,
                                op1=mybir.AluOpType.mult,
                            )
                            nc.tensor.matmul(
                                out=psum[:],
                                lhsT=s_t[:],
                                rhs=g[:, jloc, 0:F_OUT],
                                start=(k == 0),
                                stop=(k == len(tl) - 1),
                            )
                        nc.vector.tensor_copy(outsb[:], psum[:])
                    nc.sync.dma_start(out_d[w * P : (w + 1) * P, :], outsb[:])

    nc.compile()
    return nc


def kernel(x, edge_index, num_nodes):
    sched, x_lo, x_hi, idx_packed, seg_cols, norm_cols = _preprocess(
        x, edge_index, num_nodes
    )
    nc = _build_program(sched)

    in_maps = [
        {
            "x_lo": x_lo,
            "x_hi": x_hi,
            "idx": idx_packed[c],
            "seg": seg_cols[c],
            "norm": norm_cols[c],
        }
        for c in range(N_CORES)
    ]
    res = bass_utils.run_bass_kernel_spmd(
        nc,
        in_maps,
        core_ids=list(range(N_CORES)),
        trace=_TRACE["on"],
        **_TRACE["kwargs"],
    )
    _LAST["results"] = res

    n = sched["n"]
    order_nodes = sched["order_nodes"]
    out = np.empty((n, F_OUT), dtype=np.float32)
    for c in range(N_CORES):
        ids = order_nodes[c::N_CORES]  # node of (core c, pos p) at rank p*8+c
        out[ids] = res.results[c]["out"][: ids.size]
    return out
